# revision 48
# baseline (speedup 1.0000x reference)
"""AlphaStock Trainium2 kernel: 2-layer LSTM + history attention + CAAN.

Data-parallel over batch: 8 cores x 4 batch elems (512 sequences each).
LSTM in transposed-gate layout: gates in PSUM as (gate_dim, seq), h/c as
(hidden, seq). All 4 gates of a layer go through ONE sigmoid ACTIVATE
(g-gate weights pre-scaled x2 host-side; tanh(g) = 2*sigmoid(2g) - 1 is
fixed up on the vector engine), halving ACT instruction overhead. The
two layers are processed skewed (L2 one step behind L1) so PE/ACT/DVE
overlap and the PE never idles long enough to be HAM-throttled.
History attention batches tanh/exp over pairs of timesteps; softmax
denominator comes from tiny K=8 matmuls over a DMA-transposed exp
table. CAAN runs stage-major across the 4 batch elems with rank-gate
tables precomputed up front (which also warms the PE). LayerNorm rstd
uses a DVE quake-rsqrt (bitcast + Newton) and the final sigmoid goes
through exp + reciprocal, so the whole kernel needs only two ACT
table sets (sigmoid_and_others, exp_and_others).
"""

from contextlib import ExitStack

import ml_dtypes
import numpy as np

import concourse.bass as bass
import concourse.bacc as bacc
import concourse.tile as tile
from concourse import mybir
from concourse.bass_utils import run_bass_kernel_spmd
from concourse.masks import make_identity

N_CORES = 8
B, A, T, D, H, ATTN = 32, 128, 96, 16, 128, 64
MAX_DIST, EMB = 50, 32
BPC = B // N_CORES  # batch elems per core
S = BPC * A  # sequences per core = 512
G4 = 4 * H  # 512 gate dims

F32 = mybir.dt.float32
BF16 = mybir.dt.bfloat16
F8 = mybir.dt.float8e4
I32 = mybir.dt.int32
AF = mybir.ActivationFunctionType
OP = mybir.AluOpType
DR = mybir.MatmulPerfMode.DoubleRow

BF = ml_dtypes.bfloat16
E4 = ml_dtypes.float8_e4m3fn

_cache = {}


def _bc_ap(dram_handle, row_elems, row_idx, nrows=128):
    """DRAM row -> broadcast AP replicating it across `nrows` partitions."""
    return bass.AP(
        tensor=dram_handle,
        offset=row_idx * row_elems,
        ap=[[0, nrows], [1, row_elems]],
    )


def _rsqrt_quake(nc, pool, v, n):
    """out = v**-0.5 on DVE via quake bit-trick + 2 Newton iters.

    v: [A, n] fp32 SBUF AP (must be positive). Returns [A, n] fp32 tile.
    """
    ih = pool.tile([A, n], I32, tag="qk_ih")
    nc.vector.tensor_scalar(
        out=ih[:], in0=v.bitcast(I32), scalar1=1, scalar2=None,
        op0=OP.logical_shift_right)
    im = pool.tile([A, n], I32, tag="qk_im")
    nc.vector.tensor_scalar(
        out=im[:], in0=ih[:], scalar1=-1, scalar2=0x5F3759DF,
        op0=OP.mult, op1=OP.add)
    y = im[:].bitcast(F32)
    yy = pool.tile([A, n], F32, tag="qk_yy")
    hv = pool.tile([A, n], F32, tag="qk_hv")
    cc = pool.tile([A, n], F32, tag="qk_cc")
    y1 = pool.tile([A, n], F32, tag="qk_y1")
    for it in range(2):
        nc.vector.tensor_mul(yy[:], y, y)
        nc.vector.tensor_mul(hv[:], yy[:], v)
        nc.vector.tensor_scalar(
            out=cc[:], in0=hv[:], scalar1=-0.5, scalar2=1.5,
            op0=OP.mult, op1=OP.add)
        dst = y1 if it == 0 else yy
        nc.vector.tensor_mul(dst[:], y, cc[:])
        y = dst[:]
    return y


def _build(has_b0, has_b1, has_bv, has_f1b, has_f2b):
    nc = bacc.Bacc("TRN2", target_bir_lowering=False, debug=False,
                   num_devices=N_CORES)

    # ---- DRAM parameters (per-core shards / replicated weights) ----
    x_d = nc.dram_tensor("x", [T, D, S], F8, kind="ExternalInput")
    # fp8 DoubleRow weight packs: [K, gate, ktile(2), M]
    wa_d = nc.dram_tensor("wa", [A, 4 * 2 * H], F8, kind="ExternalInput")
    wb_d = nc.dram_tensor("wb", [A, 4 * 2 * H], F8, kind="ExternalInput")
    whb_d = nc.dram_tensor("whb", [A, 4 * H], BF16, kind="ExternalInput")
    b0_d = nc.dram_tensor("b0", [1, G4], BF16, kind="ExternalInput")
    b1_d = nc.dram_tensor("b1", [1, G4], BF16, kind="ExternalInput")
    aw1_d = nc.dram_tensor("aw1", [H, H], BF16, kind="ExternalInput")
    aw2_d = nc.dram_tensor("aw2", [H, H], BF16, kind="ExternalInput")
    awv_d = nc.dram_tensor("awv", [H, 1], BF16, kind="ExternalInput")
    ln1g_d = nc.dram_tensor("ln1g", [1, H], F32, kind="ExternalInput")
    ln1b_d = nc.dram_tensor("ln1b", [1, H], F32, kind="ExternalInput")
    projw_d = nc.dram_tensor("projw", [H, ATTN], BF16, kind="ExternalInput")
    projb_d = nc.dram_tensor("projb", [ATTN, 1], F32, kind="ExternalInput")
    wq_d = nc.dram_tensor("wq", [ATTN, ATTN], BF16, kind="ExternalInput")
    bq_d = nc.dram_tensor("bq", [ATTN, 1], F32, kind="ExternalInput")
    wk_d = nc.dram_tensor("wk", [ATTN, ATTN], BF16, kind="ExternalInput")
    bk_d = nc.dram_tensor("bk", [ATTN, 1], F32, kind="ExternalInput")
    wv_d = nc.dram_tensor("wv", [ATTN, ATTN], BF16, kind="ExternalInput")
    bv_d = nc.dram_tensor("bv", [1, ATTN], BF16, kind="ExternalInput")
    gmat_d = nc.dram_tensor("gmat", [A, A], BF16, kind="ExternalInput")
    iota_d = nc.dram_tensor("iotap", [A, A], I32, kind="ExternalInput")
    ranks_d = nc.dram_tensor("ranks", [BPC, A], I32, kind="ExternalInput")
    ff1_d = nc.dram_tensor("ff1", [ATTN, 2 * ATTN], BF16, kind="ExternalInput")
    ff1b_d = nc.dram_tensor("ff1b", [1, 2 * ATTN], BF16, kind="ExternalInput")
    ff2_d = nc.dram_tensor("ff2", [2 * ATTN, ATTN], BF16, kind="ExternalInput")
    ff2b_d = nc.dram_tensor("ff2b", [1, ATTN], BF16, kind="ExternalInput")
    ln2g_d = nc.dram_tensor("ln2g", [1, ATTN], F32, kind="ExternalInput")
    ln2b_d = nc.dram_tensor("ln2b", [1, ATTN], F32, kind="ExternalInput")
    sp1_d = nc.dram_tensor("sp1", [ATTN, 32], BF16, kind="ExternalInput")
    sp1b_d = nc.dram_tensor("sp1b", [32, 1], F32, kind="ExternalInput")
    sp2_d = nc.dram_tensor("sp2", [32, 1], BF16, kind="ExternalInput")
    nsp2b_d = nc.dram_tensor("nsp2b", [1, 1], F32, kind="ExternalInput")
    out_d = nc.dram_tensor("out", [BPC, A], F32, kind="ExternalOutput")

    with tile.TileContext(nc) as tc, ExitStack() as ctx:
        consts = ctx.enter_context(tc.tile_pool(name="consts", bufs=1))

        def load(dram, shape, dtype, tag):
            t = consts.tile(shape, dtype, tag=tag)
            nc.sync.dma_start(out=t[:], in_=dram.ap())
            return t

        wa = load(wa_d, [A, 4, 2, H], F8, "wa")
        wb = load(wb_d, [A, 4, 2, H], F8, "wb")
        whb = load(whb_d, [A, 4, H], BF16, "whb")
        b0 = load(b0_d, [1, G4], BF16, "b0") if has_b0 else None
        b1 = load(b1_d, [1, G4], BF16, "b1") if has_b1 else None
        aw1 = load(aw1_d, [H, H], BF16, "aw1")
        aw2 = load(aw2_d, [H, H], BF16, "aw2")
        awv = load(awv_d, [H, 1], BF16, "awv")
        projw = load(projw_d, [H, ATTN], BF16, "projw")
        projb = load(projb_d, [ATTN, 1], F32, "projb")
        wq = load(wq_d, [ATTN, ATTN], BF16, "wq")
        bq = load(bq_d, [ATTN, 1], F32, "bq")
        wk = load(wk_d, [ATTN, ATTN], BF16, "wk")
        bk = load(bk_d, [ATTN, 1], F32, "bk")
        wv = load(wv_d, [ATTN, ATTN], BF16, "wv")
        bv = load(bv_d, [1, ATTN], BF16, "bv") if has_bv else None
        gmat = load(gmat_d, [A, A], BF16, "gmat")
        iotap = load(iota_d, [A, A], I32, "iotap")
        ff1 = load(ff1_d, [ATTN, 2 * ATTN], BF16, "ff1")
        ff1b = load(ff1b_d, [1, 2 * ATTN], BF16, "ff1b") if has_f1b else None
        ff2 = load(ff2_d, [2 * ATTN, ATTN], BF16, "ff2")
        ff2b = load(ff2b_d, [1, ATTN], BF16, "ff2b") if has_f2b else None
        sp1 = load(sp1_d, [ATTN, 32], BF16, "sp1")
        sp1b = load(sp1b_d, [32, 1], F32, "sp1b")
        sp2 = load(sp2_d, [32, 1], BF16, "sp2")
        nsp2b = load(nsp2b_d, [1, 1], F32, "nsp2b")

        # broadcast constants (row replicated across partitions)
        gbc1 = consts.tile([A, H], F32, tag="gbc1")
        nc.sync.dma_start(out=gbc1[:], in_=_bc_ap(ln1g_d, H, 0))
        bbc1 = consts.tile([A, H], F32, tag="bbc1")
        nc.sync.dma_start(out=bbc1[:], in_=_bc_ap(ln1b_d, H, 0))
        gbc2 = consts.tile([A, ATTN], F32, tag="gbc2")
        nc.sync.dma_start(out=gbc2[:], in_=_bc_ap(ln2g_d, ATTN, 0))
        bbc2 = consts.tile([A, ATTN], F32, tag="bbc2")
        nc.sync.dma_start(out=bbc2[:], in_=_bc_ap(ln2b_d, ATTN, 0))

        ones_1_512 = consts.tile([1, S], BF16, tag="o1s")
        nc.vector.memset(ones_1_512[:], 1.0)
        ones_1_128b = consts.tile([1, A], BF16, tag="o1ab")
        nc.vector.memset(ones_1_128b[:], 1.0)
        ones_1_128f = consts.tile([1, A], F32, tag="o1af")
        nc.vector.memset(ones_1_128f[:], 1.0)
        ones_1_1b = consts.tile([1, 1], BF16, tag="o11")
        nc.vector.memset(ones_1_1b[:], 1.0)
        ident_b = consts.tile([A, A], BF16, tag="idb")
        make_identity(nc, ident_b[:])
        ident_f = consts.tile([A, A], F32, tag="idf")
        make_identity(nc, ident_f[:])

        # ------- rank-gate tables for all 4 batch elems (warms PE) -------
        caan_c = ctx.enter_context(tc.tile_pool(name="caanc", bufs=1))
        gates = []
        with tc.tile_pool(name="psg0", bufs=4, space="PSUM") as psg0:
            for b in range(BPC):
                rk = caan_c.tile([A, A], I32, tag=f"rk{b}")
                nc.sync.dma_start(out=rk[:], in_=_bc_ap(ranks_d, A, b))
                rbt = caan_c.tile([A, A], BF16, tag=f"rbt{b}")
                nc.vector.tensor_tensor(out=rbt[:], in0=iotap[:], in1=rk[:],
                                        op=OP.is_equal)
                g1p = psg0.tile([A, A], F32, tag="g1p")
                nc.tensor.matmul(g1p[:], gmat[:], rbt[:], start=True,
                                 stop=True)
                g1 = caan_c.tile([A, A], BF16, tag=f"g1{b}")
                nc.scalar.copy(g1[:], g1p[:])
                gatep = psg0.tile([A, A], F32, tag="gatep")
                nc.tensor.matmul(gatep[:], rbt[:], g1[:], start=True,
                                 stop=True)
                gate = caan_c.tile([A, A], BF16, tag=f"gate{b}")
                nc.scalar.copy(gate[:], gatep[:])
                gates.append(gate)

        # persistent big buffers
        big = ctx.enter_context(tc.tile_pool(name="big", bufs=1))
        h2 = big.tile([H, T, S], BF16, tag="h2")  # layer-2 hidden history

        xin = ctx.enter_context(tc.tile_pool(name="xin", bufs=3))
        st = ctx.enter_context(tc.tile_pool(name="st", bufs=2))
        gsb = ctx.enter_context(tc.tile_pool(name="gsb", bufs=2))

        # ---------------- Phase 1: 2-layer LSTM (skewed) ----------------
        h1_prev = st.tile([H, S], BF16, tag="h1", name="h1_z")
        c1_prev = st.tile([H, S], BF16, tag="c1", name="c1_z")
        c2_prev = st.tile([H, S], BF16, tag="c2", name="c2_z")
        h2z = consts.tile([H, S], BF16, tag="h2z")
        nc.vector.memset(h1_prev[:], 0.0)
        nc.vector.memset(c1_prev[:], 0.0)
        nc.vector.memset(c2_prev[:], 0.0)
        nc.vector.memset(h2z[:], 0.0)
        h2_prev = h2z

        def dve_gates(sg, c_prev, c_new, layer):
            """c_new = sig_f*c_prev + sig_i*tanh_g, tanh_g = 2*sig_g2 - 1.

            Gate column order is [i, f, g, o]; only cols [0, 3S) are read.
            """
            fc = gsb.tile([H, S], BF16, tag=f"fc{layer}")
            nc.vector.tensor_mul(fc[:], sg[:, S:2 * S], c_prev[:])
            tg = gsb.tile([H, S], BF16, tag=f"tg{layer}")
            nc.vector.tensor_scalar(
                out=tg[:], in0=sg[:, 2 * S:3 * S], scalar1=2.0, scalar2=1.0,
                op0=OP.mult, op1=OP.subtract)
            ig = gsb.tile([H, S], BF16, tag=f"ig{layer}")
            nc.vector.tensor_mul(ig[:], sg[:, 0:S], tg[:])
            nc.vector.tensor_add(c_new[:], ig[:], fc[:])

        rin = ctx.enter_context(tc.tile_pool(name="rin", bufs=2))

        def dr_gates(ps, pso, w, r, b, split=False):
            # gates i,f,g into ps (read by sig_ifg), o into its own tile so
            # sig_ifg does not wait on the 4th matmul (whole-tile deps).
            # split=True: two plain fp8 matmuls accumulating in PSUM so
            # k-tile0 (ready earlier) runs off the critical path.
            if split:
                for g in range(4):
                    dst = ps[:, g * S:(g + 1) * S] if g < 3 else pso[:]
                    nc.tensor.matmul(dst, w[:, g, 0, :], r[:, 0, :],
                                     start=True, stop=False)
            for g in range(4):
                dst = ps[:, g * S:(g + 1) * S] if g < 3 else pso[:]
                if split:
                    nc.tensor.matmul(dst, w[:, g, 1, :], r[:, 1, :],
                                     start=False, stop=b is None)
                else:
                    nc.tensor.matmul(dst, w[:, g, :, :], r[:], start=True,
                                     stop=b is None, perf_mode=DR)
                if b is not None:
                    nc.tensor.matmul(dst, b[:, g * H:(g + 1) * H],
                                     ones_1_512[:], start=False, stop=True)

        # Two persistent ping-pong rhs tiles for psA; partition rows D:A of
        # the x k-tile are zeroed once (their DR weights are zero, but the
        # values must stay finite) and never written again.
        rA_slots = [rin.tile([A, 2, S], F8, tag=f"rAs{i}", bufs=1,
                             name=f"rAs{i}")
                    for i in range(2)]
        for i in range(2):
            nc.vector.memset(rA_slots[i][:, 0, :], 0.0)
        with tc.tile_pool(name="psgA", bufs=1, space="PSUM") as psgA, \
                tc.tile_pool(name="psgB", bufs=1, space="PSUM") as psgB:
            # prologue: rA(0) = [x(0) | h1(-1)=0]
            rA = rA_slots[0]
            nc.sync.dma_start(out=rA[0:D, 0, :], in_=x_d.ap()[0, :, :])
            nc.vector.memset(rA[:, 1, :], 0.0)
            psA = psgA.tile([H, 3 * S], F32, tag="psA")
            psAo = psgA.tile([H, S], F32, tag="psAo")
            dr_gates(psA, psAo, wa, rA, b0, split=True)
            for t in range(T + 1):
                # L1(t) express lane: sigA -> DVE chain -> tanh -> h1.
                # L2(t-1) trails; PE work for t+1 is issued late so its
                # dependencies stagger it across the period.
                if t < T:
                    sgA = gsb.tile([H, 3 * S], BF16, tag="sgA")
                    nc.scalar.activation(sgA[:], psA[:], AF.Sigmoid)
                    sgAo = gsb.tile([H, S], BF16, tag="sgAo")
                    nc.scalar.activation(sgAo[:], psAo[:], AF.Sigmoid)
                if t >= 1:
                    sgB = gsb.tile([H, 3 * S], BF16, tag="sgB")
                    nc.scalar.activation(sgB[:], psB[:], AF.Sigmoid)
                if t < T:
                    c1_new = st.tile([H, S], BF16, tag="c1", name="c1_new")
                    dve_gates(sgA, c1_prev, c1_new, 1)
                    tc1 = gsb.tile([H, S], BF16, tag="tc1")
                    nc.scalar.activation(tc1[:], c1_new[:], AF.Tanh)
                    # h1(t) written as fp8 straight into next step's rhs
                    rA = rA_slots[(t + 1) % 2]
                    if t + 1 < T:
                        nc.sync.dma_start(out=rA[0:D, 0, :],
                                          in_=x_d.ap()[t + 1, :, :])
                    nc.vector.tensor_mul(rA[:, 1, :], sgAo[:], tc1[:])
                    c1_prev = c1_new
                if t >= 1:
                    sgBo = gsb.tile([H, S], BF16, tag="sgBo")
                    nc.scalar.activation(sgBo[:], psBo[:], AF.Sigmoid)
                    c2_new = st.tile([H, S], BF16, tag="c2", name="c2_new")
                    dve_gates(sgB, c2_prev, c2_new, 2)
                    tc2 = gsb.tile([H, S], BF16, tag="tc2")
                    nc.scalar.activation(tc2[:], c2_new[:], AF.Tanh)
                    nc.vector.tensor_mul(h2[:, t - 1, :], sgBo[:], tc2[:])
                    c2_prev = c2_new
                # PE: gates for L1(t+1), then L2(t)
                if t + 1 < T:
                    psA = psgA.tile([H, 3 * S], F32, tag="psA")
                    psAo = psgA.tile([H, S], F32, tag="psAo")
                    dr_gates(psA, psAo, wa, rA, b0, split=True)
                if t < T:
                    psB = psgB.tile([H, 3 * S], F32, tag="psB")
                    psBo = psgB.tile([H, S], F32, tag="psBo")
                    h2prev = h2[:, t - 1, :] if t >= 1 else h2z[:]
                    for g in range(4):
                        dst = psB[:, g * S:(g + 1) * S] if g < 3 else psBo[:]
                        nc.tensor.matmul(dst, wb[:, g, 0, :], rA[:, 1, :],
                                         start=True, stop=False)
                    for g in range(4):
                        dst = psB[:, g * S:(g + 1) * S] if g < 3 else psBo[:]
                        nc.tensor.matmul(dst, whb[:, g, :], h2prev,
                                         start=False, stop=b1 is None)
                        if b1 is not None:
                            nc.tensor.matmul(dst, b1[:, g * H:(g + 1) * H],
                                             ones_1_512[:], start=False,
                                             stop=True)

        # ---------------- Phase 2: history attention ----------------
        # alpha[t] = awv . tanh(aw1 @ h2[t] + aw2 @ hT); softmax over t;
        # ctx = sum_t w[t] * h2[t]. Processed in pairs of timesteps.
        ph2 = ctx.enter_context(tc.tile_pool(name="ph2", bufs=2))
        hT = h2[:, T - 1, :]
        ctxU = big.tile([H, S], F32, tag="ctxU")
        nc.vector.memset(ctxU[:], 0.0)
        with tc.tile_pool(name="psu", bufs=1, space="PSUM") as psu, \
                tc.tile_pool(name="psal", bufs=1, space="PSUM") as psal, \
                tc.tile_pool(name="pseb", bufs=2, space="PSUM") as pseb:
            den = psal.tile([1, S], F32, tag="den")
            for tp in range(T // 2):
                t0 = 2 * tp
                u = psu.tile([H, 2 * S], F32, tag="u")
                for j in range(2):
                    nc.tensor.matmul(u[:, j * S:(j + 1) * S], aw1[:],
                                     h2[:, t0 + j, :], start=True, stop=False)
                for j in range(2):
                    nc.tensor.matmul(u[:, j * S:(j + 1) * S], aw2[:], hT,
                                     start=False, stop=True)
                th = ph2.tile([H, 2 * S], BF16, tag="th")
                nc.scalar.activation(th[:], u[:], AF.Tanh)
                al = psal.tile([1, 2 * S], F32, tag="al")
                for j in range(2):
                    nc.tensor.matmul(al[:, j * S:(j + 1) * S], awv[:],
                                     th[:, j * S:(j + 1) * S], start=True,
                                     stop=True)
                et = ph2.tile([1, 2 * S], BF16, tag="et")
                nc.scalar.activation(et[:], al[:], AF.Exp)
                # weighted accumulation of h2 into ctxU + denominator
                tm0 = ph2.tile([H, S], BF16, tag="tm0")
                tm1 = ph2.tile([H, S], BF16, tag="tm1")
                ebs = []
                for j in range(2):
                    eb = pseb.tile([H, S], F32, tag="eb")
                    nc.tensor.matmul(eb[:], ones_1_128b[:],
                                     et[:, j * S:(j + 1) * S],
                                     start=True, stop=True)
                    ebs.append(eb)
                for j in range(2):
                    nc.tensor.matmul(den[:], ones_1_1b[:],
                                     et[:, j * S:(j + 1) * S],
                                     start=(t0 + j == 0),
                                     stop=(t0 + j == T - 1))
                for j in range(2):
                    nc.vector.tensor_mul((tm0 if j == 0 else tm1)[:],
                                         h2[:, t0 + j, :], ebs[j][:])
                pr = ph2.tile([H, S], BF16, tag="pr")
                nc.vector.tensor_add(pr[:], tm0[:], tm1[:])
                nc.vector.tensor_add(ctxU[:], ctxU[:], pr[:])

            recip = ph2.tile([1, S], F32, tag="recip")
            nc.vector.reciprocal(recip[:], den[:])
            rbc = psu.tile([H, S], F32, tag="u", name="rbc")
            nc.tensor.matmul(rbc[:], ones_1_128f[:], recip[:], start=True,
                             stop=True)
            nc.vector.tensor_mul(ctxU[:], ctxU[:], rbc[:])

        # LayerNorm over H per sequence -> rep chunks (seq, hid) bf16
        rep = []
        with tc.tile_pool(name="psl", bufs=4, space="PSUM") as psl:
            mvs = []
            var4 = ph2.tile([A, 4], F32, tag="var4")
            for chn in range(4):
                ctxT = psl.tile([A, H], F32, tag="ln")
                nc.tensor.transpose(ctxT[:], ctxU[:, chn * A:(chn + 1) * A],
                                    ident_f[:])
                cs = ph2.tile([A, H], F32, tag="cs", name=f"cs{chn}")
                nc.scalar.copy(cs[:], ctxT[:])
                st6 = ph2.tile([A, nc.vector.BN_STATS_DIM], F32, tag="st6")
                nc.vector.bn_stats(out=st6[:], in_=cs[:])
                mv = ph2.tile([A, nc.vector.BN_AGGR_DIM], F32, tag="mv",
                              name=f"mv{chn}")
                nc.vector.bn_aggr(out=mv[:], in_=st6[:])
                nc.vector.tensor_scalar_add(var4[:, chn:chn + 1], mv[:, 1:2],
                                            1e-5)
                mvs.append((cs, mv))
            rstd4 = _rsqrt_quake(nc, ph2, var4[:], 4)
            for chn in range(4):
                cs, mv = mvs[chn]
                tmp = ph2.tile([A, H], F32, tag="lt")
                nc.vector.tensor_scalar_sub(tmp[:], cs[:], mv[:, 0:1])
                tmp2 = ph2.tile([A, H], F32, tag="lt2")
                nc.vector.scalar_tensor_tensor(tmp2[:], tmp[:],
                                               rstd4[:, chn:chn + 1],
                                               gbc1[:], op0=OP.mult,
                                               op1=OP.mult)
                r = big.tile([A, H], BF16, tag=f"rep{chn}")
                nc.vector.tensor_add(r[:], tmp2[:], bbc1[:])
                rep.append(r)

        # ---------------- Phase 3: CAAN, stage-major over b ----------------
        caan = ctx.enter_context(tc.tile_pool(name="caan", bufs=2))
        with tc.tile_pool(name="psc", bufs=8, space="PSUM") as psc:
            xpT, qT, kT, vb = [], [], [], []
            for b in range(BPC):
                rT = psc.tile([A, A], BF16, tag="c", name=f"rT{b}")
                nc.tensor.transpose(rT[:], rep[b][:], ident_b[:])
                rTs = caan.tile([A, A], BF16, tag=f"rTs{b}")
                nc.vector.tensor_copy(rTs[:], rT[:])
                xpp = psc.tile([ATTN, A], F32, tag="c", name=f"xpp{b}")
                nc.tensor.matmul(xpp[:], projw[:], rTs[:], start=True,
                                 stop=True)
                xt = caan.tile([ATTN, A], BF16, tag=f"xpT{b}")
                nc.scalar.activation(xt[:], xpp[:], AF.Identity,
                                     bias=projb[:])
                xpT.append(xt)
            for b in range(BPC):
                qp = psc.tile([ATTN, A], F32, tag="c", name=f"qp{b}")
                nc.tensor.matmul(qp[:], wq[:], xpT[b][:], start=True,
                                 stop=True)
                qt = caan.tile([ATTN, A], BF16, tag=f"qT{b}")
                nc.scalar.activation(qt[:], qp[:], AF.Identity, bias=bq[:])
                qT.append(qt)
                kp = psc.tile([ATTN, A], F32, tag="c", name=f"kp{b}")
                nc.tensor.matmul(kp[:], wk[:], xpT[b][:], start=True,
                                 stop=True)
                kt = caan.tile([ATTN, A], BF16, tag=f"kT{b}")
                nc.scalar.activation(kt[:], kp[:], AF.Identity, bias=bk[:])
                kT.append(kt)
                vp = psc.tile([A, ATTN], F32, tag="c", name=f"vp{b}")
                nc.tensor.matmul(vp[:], xpT[b][:], wv[:], start=True,
                                 stop=bv is None)
                if bv is not None:
                    nc.tensor.matmul(vp[:], ones_1_128b[:], bv[:],
                                     start=False, stop=True)
                v = caan.tile([A, ATTN], BF16, tag=f"v{b}")
                nc.vector.tensor_copy(v[:], vp[:])
                vb.append(v)
            aoT = []
            for b in range(BPC):
                sc = psc.tile([A, A], F32, tag="c", name=f"sc{b}")
                nc.tensor.matmul(sc[:], qT[b][:], kT[b][:], start=True,
                                 stop=True)
                sg = caan.tile([A, A], F32, tag="sg")
                nc.vector.scalar_tensor_tensor(sg[:], sc[:],
                                               1.0 / np.sqrt(ATTN),
                                               gates[b][:],
                                               op0=OP.mult, op1=OP.mult)
                asum = caan.tile([A, 1], F32, tag="asum")
                ae = caan.tile([A, A], F32, tag="ae")
                nc.scalar.activation(ae[:], sg[:], AF.Exp, accum_out=asum[:])
                arec = caan.tile([A, 1], F32, tag="arec")
                nc.vector.reciprocal(arec[:], asum[:])
                attn = caan.tile([A, A], BF16, tag="attn")
                nc.vector.tensor_scalar_mul(attn[:], ae[:], arec[:])
                atp = psc.tile([A, A], BF16, tag="c", name=f"atp{b}")
                nc.tensor.transpose(atp[:], attn[:], ident_b[:])
                attnT = caan.tile([A, A], BF16, tag="attnT")
                nc.vector.tensor_copy(attnT[:], atp[:])
                aop = psc.tile([ATTN, A], F32, tag="c", name=f"aop{b}")
                nc.tensor.matmul(aop[:], vb[b][:], attnT[:], start=True,
                                 stop=True)
                at = caan.tile([ATTN, A], BF16, tag=f"aoT{b}")
                nc.vector.tensor_copy(at[:], aop[:])
                aoT.append(at)
            # feed-forward + LN2 (rstd batched over b)
            f2s = []
            var4b = caan.tile([A, 4], F32, tag="var4b")
            for b in range(BPC):
                h1p = psc.tile([A, 2 * ATTN], F32, tag="c", name=f"h1p{b}")
                nc.tensor.matmul(h1p[:], aoT[b][:], ff1[:], start=True,
                                 stop=ff1b is None)
                if ff1b is not None:
                    nc.tensor.matmul(h1p[:], ones_1_128b[:], ff1b[:],
                                     start=False, stop=True)
                h1c = caan.tile([A, 2 * ATTN], BF16, tag="h1c")
                nc.scalar.activation(h1c[:], h1p[:], AF.Relu)
                h1tp = psc.tile([2 * ATTN, A], BF16, tag="c",
                                name=f"h1tp{b}")
                nc.tensor.transpose(h1tp[:], h1c[:], ident_b[:])
                h1T = caan.tile([2 * ATTN, A], BF16, tag="h1T")
                nc.vector.tensor_copy(h1T[:], h1tp[:])
                f2p = psc.tile([A, ATTN], F32, tag="c", name=f"f2p{b}")
                nc.tensor.matmul(f2p[:], h1T[:], ff2[:], start=True,
                                 stop=ff2b is None)
                if ff2b is not None:
                    nc.tensor.matmul(f2p[:], ones_1_128b[:], ff2b[:],
                                     start=False, stop=True)
                f2 = caan.tile([A, ATTN], F32, tag=f"f2{b}")
                nc.vector.tensor_copy(f2[:], f2p[:])
                st6b = caan.tile([A, nc.vector.BN_STATS_DIM], F32,
                                 tag="st6b")
                nc.vector.bn_stats(out=st6b[:], in_=f2[:])
                mvb = caan.tile([A, nc.vector.BN_AGGR_DIM], F32,
                                tag=f"mvb{b}")
                nc.vector.bn_aggr(out=mvb[:], in_=st6b[:])
                nc.vector.tensor_scalar_add(var4b[:, b:b + 1], mvb[:, 1:2],
                                            1e-5)
                f2s.append((f2, mvb))
            rstd4b = _rsqrt_quake(nc, caan, var4b[:], 4)
            for b in range(BPC):
                f2, mvb = f2s[b]
                lt = caan.tile([A, ATTN], F32, tag="ltb")
                nc.vector.tensor_scalar_sub(lt[:], f2[:], mvb[:, 0:1])
                lt2 = caan.tile([A, ATTN], F32, tag="ltb2")
                nc.vector.scalar_tensor_tensor(lt2[:], lt[:],
                                               rstd4b[:, b:b + 1],
                                               gbc2[:], op0=OP.mult,
                                               op1=OP.mult)
                ffo = caan.tile([A, ATTN], BF16, tag="ffo")
                nc.vector.tensor_add(ffo[:], lt2[:], bbc2[:])
                # scorer: sigmoid(sp2 @ relu(sp1 @ ff + b1) + b2) via exp
                fftp = psc.tile([ATTN, A], BF16, tag="c", name=f"fftp{b}")
                nc.tensor.transpose(fftp[:], ffo[:], ident_b[:])
                ffT = caan.tile([ATTN, A], BF16, tag="ffT")
                nc.vector.tensor_copy(ffT[:], fftp[:])
                s1p = psc.tile([32, A], F32, tag="c", name=f"s1p{b}")
                nc.tensor.matmul(s1p[:], sp1[:], ffT[:], start=True,
                                 stop=True)
                s1 = caan.tile([32, A], BF16, tag="s1")
                nc.scalar.activation(s1[:], s1p[:], AF.Relu, bias=sp1b[:])
                s2p = psc.tile([1, A], F32, tag="c", name=f"s2p{b}")
                nc.tensor.matmul(s2p[:], sp2[:], s1[:], start=True, stop=True)
                # sigmoid(z) = 1 / (1 + exp(-z)); nsp2b = -sp2_bias
                en = caan.tile([1, A], F32, tag="en")
                nc.scalar.activation(en[:], s2p[:], AF.Exp, bias=nsp2b[:],
                                     scale=-1.0)
                ep1 = caan.tile([1, A], F32, tag="ep1")
                nc.vector.tensor_scalar_add(ep1[:], en[:], 1.0)
                s2 = caan.tile([1, A], F32, tag="s2")
                nc.vector.reciprocal(s2[:], ep1[:])
                nc.sync.dma_start(out=out_d.ap()[b:b + 1, :], in_=s2[:])

    nc.compile()
    return nc


def _reord(w):
    """PyTorch gate order i,f,g,o kept as i,f,g,o (on last axis), with the
    g-gate block scaled x2 (tanh(g) = 2*sigmoid(2g) - 1)."""
    i, f, g, o = np.split(w, 4, axis=-1)
    return np.concatenate([i, f, 2.0 * g, o], axis=-1)


def kernel(**inp):
    x = np.asarray(inp["x"], np.float32)
    ranks = np.asarray(inp["ranks"], np.int32)

    def bf(a):
        return np.ascontiguousarray(np.asarray(a, np.float32).astype(BF))

    def f8(a):
        return np.clip(np.asarray(a, np.float32), -240, 240).astype(E4)

    w0t = _reord(np.asarray(inp["W_ih0"], np.float32).T)
    whh0 = _reord(np.asarray(inp["W_hh0"], np.float32).T)
    wih1 = _reord(np.asarray(inp["W_ih1"], np.float32).T)
    whh1 = _reord(np.asarray(inp["W_hh1"], np.float32).T)
    # fp8 DoubleRow packs [K, gate, ktile, M]: psA = wih0@x + whh0@h1,
    # psB = wih1@h1 + whh1@h2
    waf = np.zeros((A, 4, 2, H), np.float32)
    wbf = np.zeros((A, 4, 2, H), np.float32)
    for g in range(4):
        waf[0:D, g, 0, :] = w0t[:, g * H:(g + 1) * H]
        waf[:, g, 1, :] = whh0[:, g * H:(g + 1) * H]
        wbf[:, g, 0, :] = wih1[:, g * H:(g + 1) * H]
        wbf[:, g, 1, :] = whh1[:, g * H:(g + 1) * H]
    wa = np.ascontiguousarray(f8(waf).reshape(A, 4 * 2 * H))
    wb = np.ascontiguousarray(f8(wbf).reshape(A, 4 * 2 * H))
    whb = bf(whh1)
    b0v = np.asarray(inp["b_ih0"], np.float32) + np.asarray(inp["b_hh0"],
                                                            np.float32)
    b1v = np.asarray(inp["b_ih1"], np.float32) + np.asarray(inp["b_hh1"],
                                                            np.float32)
    b0 = bf(_reord(b0v)[None, :])
    b1 = bf(_reord(b1v)[None, :])

    # host-precomputed rank-distance gate table: gmat[p, q] = gate(|p-q|)
    emb = np.asarray(inp["rank_emb"], np.float32)
    rw1 = np.asarray(inp["rw1_W"], np.float32)
    rw1b = np.asarray(inp["rw1_b"], np.float32)
    rw2 = np.asarray(inp["rw2_W"], np.float32)
    gv = 1.0 / (1.0 + np.exp(-(np.maximum(emb @ rw1 + rw1b, 0.0) @ rw2)))
    pq = np.abs(np.arange(A)[:, None] - np.arange(A)[None, :])
    gmat = bf(gv[np.clip(pq, 0, MAX_DIST)])
    iotap = np.ascontiguousarray(
        np.broadcast_to(np.arange(A, dtype=np.int32)[:, None], (A, A)))

    has_b0 = bool(np.any(b0v))
    has_b1 = bool(np.any(b1v))
    has_bv = bool(np.any(np.asarray(inp["bv"], np.float32)))
    has_f1b = bool(np.any(np.asarray(inp["ff1_b"], np.float32)))
    has_f2b = bool(np.any(np.asarray(inp["ff2_b"], np.float32)))
    ck = (has_b0, has_b1, has_bv, has_f1b, has_f2b)
    if ck not in _cache:
        _cache[ck] = _build(*ck)
    nc = _cache[ck]

    shared = dict(
        wa=wa, wb=wb, whb=whb, b0=b0, b1=b1,
        aw1=bf(inp["attn_W1"]), aw2=bf(inp["attn_W2"]),
        awv=bf(np.asarray(inp["attn_w"], np.float32)[:, None]),
        ln1g=np.asarray(inp["ln1_g"], np.float32)[None, :].copy(),
        ln1b=np.asarray(inp["ln1_b"], np.float32)[None, :].copy(),
        projw=bf(inp["proj_W"]),
        projb=np.asarray(inp["proj_b"], np.float32)[:, None].copy(),
        wq=bf(inp["Wq"]), bq=np.asarray(inp["bq"], np.float32)[:, None].copy(),
        wk=bf(inp["Wk"]), bk=np.asarray(inp["bk"], np.float32)[:, None].copy(),
        wv=bf(inp["Wv"]), bv=bf(np.asarray(inp["bv"], np.float32)[None, :]),
        gmat=gmat, iotap=iotap,
        ff1=bf(inp["ff1_W"]),
        ff1b=bf(np.asarray(inp["ff1_b"], np.float32)[None, :]),
        ff2=bf(inp["ff2_W"]),
        ff2b=bf(np.asarray(inp["ff2_b"], np.float32)[None, :]),
        ln2g=np.asarray(inp["ln2_g"], np.float32)[None, :].copy(),
        ln2b=np.asarray(inp["ln2_b"], np.float32)[None, :].copy(),
        sp1=bf(inp["sp1_W"]),
        sp1b=np.asarray(inp["sp1_b"], np.float32)[:, None].copy(),
        sp2=bf(inp["sp2_W"]),
        nsp2b=(-np.asarray(inp["sp2_b"], np.float32))[None, :].copy(),
    )

    in_maps = []
    for c in range(N_CORES):
        xc = x[c * BPC:(c + 1) * BPC].reshape(S, T, D).transpose(1, 2, 0)
        m = dict(shared)
        m["x"] = np.ascontiguousarray(f8(xc))
        m["ranks"] = np.ascontiguousarray(ranks[c * BPC:(c + 1) * BPC])
        in_maps.append(m)

    global _last_in_maps
    _last_in_maps = in_maps
    res = run_bass_kernel_spmd(nc, in_maps, core_ids=list(range(N_CORES)))
    out = np.concatenate([res.results[c]["out"] for c in range(N_CORES)],
                         axis=0)
    return out.astype(np.float32)


# revision 49
# speedup vs baseline: 1.0967x; 1.0967x over previous
"""AlphaStock Trainium2 kernel: 2-layer LSTM + history attention + CAAN.

Data-parallel over batch: 8 cores x 4 batch elems (512 sequences each).
LSTM in transposed-gate layout: gates in PSUM as (gate_dim, seq), h/c as
(hidden, seq). All 4 gates of a layer go through ONE sigmoid ACTIVATE
(g-gate weights pre-scaled x2 host-side; tanh(g) = 2*sigmoid(2g) - 1 is
fixed up on the vector engine), halving ACT instruction overhead. The
two layers are processed skewed (L2 one step behind L1) so PE/ACT/DVE
overlap and the PE never idles long enough to be HAM-throttled.
History attention batches tanh/exp over pairs of timesteps; softmax
denominator comes from tiny K=8 matmuls over a DMA-transposed exp
table. CAAN runs stage-major across the 4 batch elems with rank-gate
tables precomputed up front (which also warms the PE). LayerNorm rstd
uses a DVE quake-rsqrt (bitcast + Newton) and the final sigmoid goes
through exp + reciprocal, so the whole kernel needs only two ACT
table sets (sigmoid_and_others, exp_and_others).
"""

from contextlib import ExitStack

import ml_dtypes
import numpy as np

import concourse.bass as bass
import concourse.bacc as bacc
import concourse.tile as tile
from concourse import mybir
from concourse.bass_utils import run_bass_kernel_spmd
from concourse.masks import make_identity

N_CORES = 8
B, A, T, D, H, ATTN = 32, 128, 96, 16, 128, 64
MAX_DIST, EMB = 50, 32
BPC = B // N_CORES  # batch elems per core
S = BPC * A  # sequences per core = 512
G4 = 4 * H  # 512 gate dims

F32 = mybir.dt.float32
BF16 = mybir.dt.bfloat16
F8 = mybir.dt.float8e4
I32 = mybir.dt.int32
AF = mybir.ActivationFunctionType
OP = mybir.AluOpType
DR = mybir.MatmulPerfMode.DoubleRow

BF = ml_dtypes.bfloat16
E4 = ml_dtypes.float8_e4m3fn

_cache = {}


def _bc_ap(dram_handle, row_elems, row_idx, nrows=128):
    """DRAM row -> broadcast AP replicating it across `nrows` partitions."""
    return bass.AP(
        tensor=dram_handle,
        offset=row_idx * row_elems,
        ap=[[0, nrows], [1, row_elems]],
    )


def _rsqrt_quake(nc, pool, v, n):
    """out = v**-0.5 on DVE via quake bit-trick + 2 Newton iters.

    v: [A, n] fp32 SBUF AP (must be positive). Returns [A, n] fp32 tile.
    """
    ih = pool.tile([A, n], I32, tag="qk_ih")
    nc.vector.tensor_scalar(
        out=ih[:], in0=v.bitcast(I32), scalar1=1, scalar2=None,
        op0=OP.logical_shift_right)
    im = pool.tile([A, n], I32, tag="qk_im")
    nc.vector.tensor_scalar(
        out=im[:], in0=ih[:], scalar1=-1, scalar2=0x5F3759DF,
        op0=OP.mult, op1=OP.add)
    y = im[:].bitcast(F32)
    yy = pool.tile([A, n], F32, tag="qk_yy")
    hv = pool.tile([A, n], F32, tag="qk_hv")
    cc = pool.tile([A, n], F32, tag="qk_cc")
    y1 = pool.tile([A, n], F32, tag="qk_y1")
    for it in range(2):
        nc.vector.tensor_mul(yy[:], y, y)
        nc.vector.tensor_mul(hv[:], yy[:], v)
        nc.vector.tensor_scalar(
            out=cc[:], in0=hv[:], scalar1=-0.5, scalar2=1.5,
            op0=OP.mult, op1=OP.add)
        dst = y1 if it == 0 else yy
        nc.vector.tensor_mul(dst[:], y, cc[:])
        y = dst[:]
    return y


def _build(has_b0, has_b1, has_bv, has_f1b, has_f2b):
    nc = bacc.Bacc("TRN2", target_bir_lowering=False, debug=False,
                   num_devices=N_CORES)

    # ---- DRAM parameters (per-core shards / replicated weights) ----
    x_d = nc.dram_tensor("x", [T, D, S], F8, kind="ExternalInput")
    # fp8 DoubleRow weight packs: [K, gate, ktile(2), M]
    wa_d = nc.dram_tensor("wa", [A, 4 * 2 * H], F8, kind="ExternalInput")
    wb_d = nc.dram_tensor("wb", [A, 4 * 2 * H], F8, kind="ExternalInput")
    b0_d = nc.dram_tensor("b0", [1, G4], BF16, kind="ExternalInput")
    b1_d = nc.dram_tensor("b1", [1, G4], BF16, kind="ExternalInput")
    aw1_d = nc.dram_tensor("aw1", [H, H], BF16, kind="ExternalInput")
    aw2_d = nc.dram_tensor("aw2", [H, H], BF16, kind="ExternalInput")
    awv_d = nc.dram_tensor("awv", [H, 1], BF16, kind="ExternalInput")
    ln1g_d = nc.dram_tensor("ln1g", [1, H], F32, kind="ExternalInput")
    ln1b_d = nc.dram_tensor("ln1b", [1, H], F32, kind="ExternalInput")
    projw_d = nc.dram_tensor("projw", [H, ATTN], BF16, kind="ExternalInput")
    projb_d = nc.dram_tensor("projb", [ATTN, 1], F32, kind="ExternalInput")
    wq_d = nc.dram_tensor("wq", [ATTN, ATTN], BF16, kind="ExternalInput")
    bq_d = nc.dram_tensor("bq", [ATTN, 1], F32, kind="ExternalInput")
    wk_d = nc.dram_tensor("wk", [ATTN, ATTN], BF16, kind="ExternalInput")
    bk_d = nc.dram_tensor("bk", [ATTN, 1], F32, kind="ExternalInput")
    wv_d = nc.dram_tensor("wv", [ATTN, ATTN], BF16, kind="ExternalInput")
    bv_d = nc.dram_tensor("bv", [1, ATTN], BF16, kind="ExternalInput")
    gmat_d = nc.dram_tensor("gmat", [A, A], BF16, kind="ExternalInput")
    iota_d = nc.dram_tensor("iotap", [A, A], I32, kind="ExternalInput")
    ranks_d = nc.dram_tensor("ranks", [BPC, A], I32, kind="ExternalInput")
    ff1_d = nc.dram_tensor("ff1", [ATTN, 2 * ATTN], BF16, kind="ExternalInput")
    ff1b_d = nc.dram_tensor("ff1b", [1, 2 * ATTN], BF16, kind="ExternalInput")
    ff2_d = nc.dram_tensor("ff2", [2 * ATTN, ATTN], BF16, kind="ExternalInput")
    ff2b_d = nc.dram_tensor("ff2b", [1, ATTN], BF16, kind="ExternalInput")
    ln2g_d = nc.dram_tensor("ln2g", [1, ATTN], F32, kind="ExternalInput")
    ln2b_d = nc.dram_tensor("ln2b", [1, ATTN], F32, kind="ExternalInput")
    sp1_d = nc.dram_tensor("sp1", [ATTN, 32], BF16, kind="ExternalInput")
    sp1b_d = nc.dram_tensor("sp1b", [32, 1], F32, kind="ExternalInput")
    sp2_d = nc.dram_tensor("sp2", [32, 1], BF16, kind="ExternalInput")
    nsp2b_d = nc.dram_tensor("nsp2b", [1, 1], F32, kind="ExternalInput")
    out_d = nc.dram_tensor("out", [BPC, A], F32, kind="ExternalOutput")

    with tile.TileContext(nc) as tc, ExitStack() as ctx:
        consts = ctx.enter_context(tc.tile_pool(name="consts", bufs=1))

        def load(dram, shape, dtype, tag):
            t = consts.tile(shape, dtype, tag=tag)
            nc.sync.dma_start(out=t[:], in_=dram.ap())
            return t

        wa = load(wa_d, [A, 4, 2, H], F8, "wa")
        wb = load(wb_d, [A, 4, 2, H], F8, "wb")
        b0 = load(b0_d, [1, G4], BF16, "b0") if has_b0 else None
        b1 = load(b1_d, [1, G4], BF16, "b1") if has_b1 else None
        aw1 = load(aw1_d, [H, H], BF16, "aw1")
        aw2 = load(aw2_d, [H, H], BF16, "aw2")
        awv = load(awv_d, [H, 1], BF16, "awv")
        projw = load(projw_d, [H, ATTN], BF16, "projw")
        projb = load(projb_d, [ATTN, 1], F32, "projb")
        wq = load(wq_d, [ATTN, ATTN], BF16, "wq")
        bq = load(bq_d, [ATTN, 1], F32, "bq")
        wk = load(wk_d, [ATTN, ATTN], BF16, "wk")
        bk = load(bk_d, [ATTN, 1], F32, "bk")
        wv = load(wv_d, [ATTN, ATTN], BF16, "wv")
        bv = load(bv_d, [1, ATTN], BF16, "bv") if has_bv else None
        gmat = load(gmat_d, [A, A], BF16, "gmat")
        iotap = load(iota_d, [A, A], I32, "iotap")
        ff1 = load(ff1_d, [ATTN, 2 * ATTN], BF16, "ff1")
        ff1b = load(ff1b_d, [1, 2 * ATTN], BF16, "ff1b") if has_f1b else None
        ff2 = load(ff2_d, [2 * ATTN, ATTN], BF16, "ff2")
        ff2b = load(ff2b_d, [1, ATTN], BF16, "ff2b") if has_f2b else None
        sp1 = load(sp1_d, [ATTN, 32], BF16, "sp1")
        sp1b = load(sp1b_d, [32, 1], F32, "sp1b")
        sp2 = load(sp2_d, [32, 1], BF16, "sp2")
        nsp2b = load(nsp2b_d, [1, 1], F32, "nsp2b")

        # broadcast constants (row replicated across partitions)
        gbc1 = consts.tile([A, H], F32, tag="gbc1")
        nc.sync.dma_start(out=gbc1[:], in_=_bc_ap(ln1g_d, H, 0))
        bbc1 = consts.tile([A, H], F32, tag="bbc1")
        nc.sync.dma_start(out=bbc1[:], in_=_bc_ap(ln1b_d, H, 0))
        gbc2 = consts.tile([A, ATTN], F32, tag="gbc2")
        nc.sync.dma_start(out=gbc2[:], in_=_bc_ap(ln2g_d, ATTN, 0))
        bbc2 = consts.tile([A, ATTN], F32, tag="bbc2")
        nc.sync.dma_start(out=bbc2[:], in_=_bc_ap(ln2b_d, ATTN, 0))

        ones_1_512 = consts.tile([1, S], BF16, tag="o1s")
        nc.vector.memset(ones_1_512[:], 1.0)
        ones_1_128b = consts.tile([1, A], BF16, tag="o1ab")
        nc.vector.memset(ones_1_128b[:], 1.0)
        ones_1_128f = consts.tile([1, A], F32, tag="o1af")
        nc.vector.memset(ones_1_128f[:], 1.0)
        ones_1_1b = consts.tile([1, 1], BF16, tag="o11")
        nc.vector.memset(ones_1_1b[:], 1.0)
        ident_b = consts.tile([A, A], BF16, tag="idb")
        make_identity(nc, ident_b[:])
        ident_f = consts.tile([A, A], F32, tag="idf")
        make_identity(nc, ident_f[:])

        # ------- rank-gate tables for all 4 batch elems (warms PE) -------
        caan_c = ctx.enter_context(tc.tile_pool(name="caanc", bufs=1))
        gates = []
        with tc.tile_pool(name="psg0", bufs=4, space="PSUM") as psg0:
            for b in range(BPC):
                rk = caan_c.tile([A, A], I32, tag=f"rk{b}")
                nc.sync.dma_start(out=rk[:], in_=_bc_ap(ranks_d, A, b))
                rbt = caan_c.tile([A, A], BF16, tag=f"rbt{b}")
                nc.vector.tensor_tensor(out=rbt[:], in0=iotap[:], in1=rk[:],
                                        op=OP.is_equal)
                g1p = psg0.tile([A, A], F32, tag="g1p")
                nc.tensor.matmul(g1p[:], gmat[:], rbt[:], start=True,
                                 stop=True)
                g1 = caan_c.tile([A, A], BF16, tag=f"g1{b}")
                nc.scalar.copy(g1[:], g1p[:])
                gatep = psg0.tile([A, A], F32, tag="gatep")
                nc.tensor.matmul(gatep[:], rbt[:], g1[:], start=True,
                                 stop=True)
                gate = caan_c.tile([A, A], BF16, tag=f"gate{b}")
                nc.scalar.copy(gate[:], gatep[:])
                gates.append(gate)

        # persistent big buffers
        big = ctx.enter_context(tc.tile_pool(name="big", bufs=1))
        h2 = big.tile([H, T, S], BF16, tag="h2")  # layer-2 hidden history

        xin = ctx.enter_context(tc.tile_pool(name="xin", bufs=3))
        st = ctx.enter_context(tc.tile_pool(name="st", bufs=2))
        gsb = ctx.enter_context(tc.tile_pool(name="gsb", bufs=2))

        # ---------------- Phase 1: 2-layer LSTM (skewed) ----------------
        h1_prev = st.tile([H, S], BF16, tag="h1", name="h1_z")
        c1_prev = st.tile([H, S], BF16, tag="c1", name="c1_z")
        c2_prev = st.tile([H, S], BF16, tag="c2", name="c2_z")
        h2z = consts.tile([H, S], BF16, tag="h2z")
        nc.vector.memset(h1_prev[:], 0.0)
        nc.vector.memset(c1_prev[:], 0.0)
        nc.vector.memset(c2_prev[:], 0.0)
        nc.vector.memset(h2z[:], 0.0)
        h2_prev = h2z

        def dve_gates(sg, c_prev, c_new, layer):
            """c_new = sig_f*c_prev + sig_i*tanh_g, tanh_g = 2*sig_g2 - 1.

            Gate column order is [i, f, g, o]; only cols [0, 3S) are read.
            """
            fc = gsb.tile([H, S], BF16, tag=f"fc{layer}")
            nc.vector.tensor_mul(fc[:], sg[:, S:2 * S], c_prev[:])
            tg = gsb.tile([H, S], BF16, tag=f"tg{layer}")
            nc.vector.tensor_scalar(
                out=tg[:], in0=sg[:, 2 * S:3 * S], scalar1=2.0, scalar2=1.0,
                op0=OP.mult, op1=OP.subtract)
            ig = gsb.tile([H, S], BF16, tag=f"ig{layer}")
            nc.vector.tensor_mul(ig[:], sg[:, 0:S], tg[:])
            nc.vector.tensor_add(c_new[:], ig[:], fc[:])

        rin = ctx.enter_context(tc.tile_pool(name="rin", bufs=2))

        def dr_gates(ps, pso, w, r, b, split=False):
            # gates i,f,g into ps (read by sig_ifg), o into its own tile so
            # sig_ifg does not wait on the 4th matmul (whole-tile deps).
            # split=True: two plain fp8 matmuls accumulating in PSUM so
            # k-tile0 (the x projection, ready early) runs off the critical
            # path; k-tile1 (recurrent) is all that follows h.
            if split:
                for g in range(4):
                    dst = ps[:, g * S:(g + 1) * S] if g < 3 else pso[:]
                    nc.tensor.matmul(dst, w[:, g, 0, :], r[:, 0, :],
                                     start=True, stop=False)
            for g in range(4):
                dst = ps[:, g * S:(g + 1) * S] if g < 3 else pso[:]
                if split:
                    nc.tensor.matmul(dst, w[:, g, 1, :], r[:, 1, :],
                                     start=False, stop=b is None)
                else:
                    nc.tensor.matmul(dst, w[:, g, :, :], r[:], start=True,
                                     stop=b is None, perf_mode=DR)
                if b is not None:
                    nc.tensor.matmul(dst, b[:, g * H:(g + 1) * H],
                                     ones_1_512[:], start=False, stop=True)

        # Two persistent ping-pong rhs tiles for psA; partition rows D:A of
        # the x k-tile are zeroed once (their DR weights are zero, but the
        # values must stay finite) and never written again.
        rA_slots = [rin.tile([A, 2, S], F8, tag=f"rAs{i}", bufs=1,
                             name=f"rAs{i}")
                    for i in range(2)]
        for i in range(2):
            nc.vector.memset(rA_slots[i][:, 0, :], 0.0)
        with tc.tile_pool(name="psgA", bufs=1, space="PSUM") as psgA, \
                tc.tile_pool(name="psgB", bufs=1, space="PSUM") as psgB:
            # prologue: rA(0) = [x(0) | h1(-1)=0]
            rA = rA_slots[0]
            nc.sync.dma_start(out=rA[0:D, 0, :], in_=x_d.ap()[0, :, :])
            nc.vector.memset(rA[:, 1, :], 0.0)
            psA = psgA.tile([H, 3 * S], F32, tag="psA")
            psAo = psgA.tile([H, S], F32, tag="psAo")
            dr_gates(psA, psAo, wa, rA, b0, split=True)
            for t in range(T + 1):
                # L1(t) express lane: sigA -> DVE chain -> tanh -> h1.
                # L2(t-1) trails; PE work for t+1 is issued late so its
                # dependencies stagger it across the period.
                if t < T:
                    sgA = gsb.tile([H, 3 * S], BF16, tag="sgA")
                    nc.scalar.activation(sgA[:], psA[:], AF.Sigmoid)
                    sgAo = gsb.tile([H, S], BF16, tag="sgAo")
                    nc.scalar.activation(sgAo[:], psAo[:], AF.Sigmoid)
                if t >= 1:
                    sgB = gsb.tile([H, 3 * S], BF16, tag="sgB")
                    nc.scalar.activation(sgB[:], psB[:], AF.Sigmoid)
                if t < T:
                    c1_new = st.tile([H, S], BF16, tag="c1", name="c1_new")
                    dve_gates(sgA, c1_prev, c1_new, 1)
                    tc1 = gsb.tile([H, S], BF16, tag="tc1")
                    nc.scalar.activation(tc1[:], c1_new[:], AF.Tanh)
                    # h1(t) written as fp8 straight into next step's rhs
                    rA = rA_slots[(t + 1) % 2]
                    if t + 1 < T:
                        nc.sync.dma_start(out=rA[0:D, 0, :],
                                          in_=x_d.ap()[t + 1, :, :])
                    nc.vector.tensor_mul(rA[:, 1, :], sgAo[:], tc1[:])
                    rB = rin.tile([A, 2, S], F8, tag="rB")
                    nc.sync.dma_start(out=rB[:, 0, :], in_=rA[:, 1, :])
                    if t == 0:
                        nc.vector.memset(rB[:, 1, :], 0.0)
                    c1_prev = c1_new
                if t >= 1:
                    sgBo = gsb.tile([H, S], BF16, tag="sgBo")
                    nc.scalar.activation(sgBo[:], psBo[:], AF.Sigmoid)
                    c2_new = st.tile([H, S], BF16, tag="c2", name="c2_new")
                    dve_gates(sgB, c2_prev, c2_new, 2)
                    tc2 = gsb.tile([H, S], BF16, tag="tc2")
                    nc.scalar.activation(tc2[:], c2_new[:], AF.Tanh)
                    nc.vector.tensor_mul(h2[:, t - 1, :], sgBo[:], tc2[:])
                    c2_prev = c2_new
                    if t < T:
                        nc.vector.tensor_copy(rB[:, 1, :], h2[:, t - 1, :])
                # PE: gates for L1(t+1), then L2(t)
                if t + 1 < T:
                    psA = psgA.tile([H, 3 * S], F32, tag="psA")
                    psAo = psgA.tile([H, S], F32, tag="psAo")
                    dr_gates(psA, psAo, wa, rA, b0, split=True)
                if t < T:
                    psB = psgB.tile([H, 3 * S], F32, tag="psB")
                    psBo = psgB.tile([H, S], F32, tag="psBo")
                    dr_gates(psB, psBo, wb, rB, b1)

        # ---------------- Phase 2: history attention ----------------
        # alpha[t] = awv . tanh(aw1 @ h2[t] + aw2 @ hT); softmax over t;
        # ctx = sum_t w[t] * h2[t]. Processed in pairs of timesteps.
        ph2 = ctx.enter_context(tc.tile_pool(name="ph2", bufs=2))
        hT = h2[:, T - 1, :]
        ctxU = big.tile([H, S], F32, tag="ctxU")
        nc.vector.memset(ctxU[:], 0.0)
        with tc.tile_pool(name="psu", bufs=1, space="PSUM") as psu, \
                tc.tile_pool(name="psal", bufs=1, space="PSUM") as psal, \
                tc.tile_pool(name="pseb", bufs=2, space="PSUM") as pseb:
            den = psal.tile([1, S], F32, tag="den")
            for tp in range(T // 2):
                t0 = 2 * tp
                u = psu.tile([H, 2 * S], F32, tag="u")
                for j in range(2):
                    nc.tensor.matmul(u[:, j * S:(j + 1) * S], aw1[:],
                                     h2[:, t0 + j, :], start=True, stop=False)
                for j in range(2):
                    nc.tensor.matmul(u[:, j * S:(j + 1) * S], aw2[:], hT,
                                     start=False, stop=True)
                th = ph2.tile([H, 2 * S], BF16, tag="th")
                nc.scalar.activation(th[:], u[:], AF.Tanh)
                al = psal.tile([1, 2 * S], F32, tag="al")
                for j in range(2):
                    nc.tensor.matmul(al[:, j * S:(j + 1) * S], awv[:],
                                     th[:, j * S:(j + 1) * S], start=True,
                                     stop=True)
                et = ph2.tile([1, 2 * S], BF16, tag="et")
                nc.scalar.activation(et[:], al[:], AF.Exp)
                # weighted accumulation of h2 into ctxU + denominator
                tm0 = ph2.tile([H, S], BF16, tag="tm0")
                tm1 = ph2.tile([H, S], BF16, tag="tm1")
                ebs = []
                for j in range(2):
                    eb = pseb.tile([H, S], F32, tag="eb")
                    nc.tensor.matmul(eb[:], ones_1_128b[:],
                                     et[:, j * S:(j + 1) * S],
                                     start=True, stop=True)
                    ebs.append(eb)
                for j in range(2):
                    nc.tensor.matmul(den[:], ones_1_1b[:],
                                     et[:, j * S:(j + 1) * S],
                                     start=(t0 + j == 0),
                                     stop=(t0 + j == T - 1))
                for j in range(2):
                    nc.vector.tensor_mul((tm0 if j == 0 else tm1)[:],
                                         h2[:, t0 + j, :], ebs[j][:])
                pr = ph2.tile([H, S], BF16, tag="pr")
                nc.vector.tensor_add(pr[:], tm0[:], tm1[:])
                nc.vector.tensor_add(ctxU[:], ctxU[:], pr[:])

            recip = ph2.tile([1, S], F32, tag="recip")
            nc.vector.reciprocal(recip[:], den[:])
            rbc = psu.tile([H, S], F32, tag="u", name="rbc")
            nc.tensor.matmul(rbc[:], ones_1_128f[:], recip[:], start=True,
                             stop=True)
            nc.vector.tensor_mul(ctxU[:], ctxU[:], rbc[:])

        # LayerNorm over H per sequence -> rep chunks (seq, hid) bf16
        rep = []
        with tc.tile_pool(name="psl", bufs=4, space="PSUM") as psl:
            mvs = []
            var4 = ph2.tile([A, 4], F32, tag="var4")
            for chn in range(4):
                ctxT = psl.tile([A, H], F32, tag="ln")
                nc.tensor.transpose(ctxT[:], ctxU[:, chn * A:(chn + 1) * A],
                                    ident_f[:])
                cs = ph2.tile([A, H], F32, tag="cs", name=f"cs{chn}")
                nc.scalar.copy(cs[:], ctxT[:])
                st6 = ph2.tile([A, nc.vector.BN_STATS_DIM], F32, tag="st6")
                nc.vector.bn_stats(out=st6[:], in_=cs[:])
                mv = ph2.tile([A, nc.vector.BN_AGGR_DIM], F32, tag="mv",
                              name=f"mv{chn}")
                nc.vector.bn_aggr(out=mv[:], in_=st6[:])
                nc.vector.tensor_scalar_add(var4[:, chn:chn + 1], mv[:, 1:2],
                                            1e-5)
                mvs.append((cs, mv))
            rstd4 = _rsqrt_quake(nc, ph2, var4[:], 4)
            for chn in range(4):
                cs, mv = mvs[chn]
                tmp = ph2.tile([A, H], F32, tag="lt")
                nc.vector.tensor_scalar_sub(tmp[:], cs[:], mv[:, 0:1])
                tmp2 = ph2.tile([A, H], F32, tag="lt2")
                nc.vector.scalar_tensor_tensor(tmp2[:], tmp[:],
                                               rstd4[:, chn:chn + 1],
                                               gbc1[:], op0=OP.mult,
                                               op1=OP.mult)
                r = big.tile([A, H], BF16, tag=f"rep{chn}")
                nc.vector.tensor_add(r[:], tmp2[:], bbc1[:])
                rep.append(r)

        # ---------------- Phase 3: CAAN, stage-major over b ----------------
        caan = ctx.enter_context(tc.tile_pool(name="caan", bufs=2))
        with tc.tile_pool(name="psc", bufs=8, space="PSUM") as psc:
            xpT, qT, kT, vb = [], [], [], []
            for b in range(BPC):
                rT = psc.tile([A, A], BF16, tag="c", name=f"rT{b}")
                nc.tensor.transpose(rT[:], rep[b][:], ident_b[:])
                rTs = caan.tile([A, A], BF16, tag=f"rTs{b}")
                nc.vector.tensor_copy(rTs[:], rT[:])
                xpp = psc.tile([ATTN, A], F32, tag="c", name=f"xpp{b}")
                nc.tensor.matmul(xpp[:], projw[:], rTs[:], start=True,
                                 stop=True)
                xt = caan.tile([ATTN, A], BF16, tag=f"xpT{b}")
                nc.scalar.activation(xt[:], xpp[:], AF.Identity,
                                     bias=projb[:])
                xpT.append(xt)
            for b in range(BPC):
                qp = psc.tile([ATTN, A], F32, tag="c", name=f"qp{b}")
                nc.tensor.matmul(qp[:], wq[:], xpT[b][:], start=True,
                                 stop=True)
                qt = caan.tile([ATTN, A], BF16, tag=f"qT{b}")
                nc.scalar.activation(qt[:], qp[:], AF.Identity, bias=bq[:])
                qT.append(qt)
                kp = psc.tile([ATTN, A], F32, tag="c", name=f"kp{b}")
                nc.tensor.matmul(kp[:], wk[:], xpT[b][:], start=True,
                                 stop=True)
                kt = caan.tile([ATTN, A], BF16, tag=f"kT{b}")
                nc.scalar.activation(kt[:], kp[:], AF.Identity, bias=bk[:])
                kT.append(kt)
                vp = psc.tile([A, ATTN], F32, tag="c", name=f"vp{b}")
                nc.tensor.matmul(vp[:], xpT[b][:], wv[:], start=True,
                                 stop=bv is None)
                if bv is not None:
                    nc.tensor.matmul(vp[:], ones_1_128b[:], bv[:],
                                     start=False, stop=True)
                v = caan.tile([A, ATTN], BF16, tag=f"v{b}")
                nc.vector.tensor_copy(v[:], vp[:])
                vb.append(v)
            aoT = []
            for b in range(BPC):
                sc = psc.tile([A, A], F32, tag="c", name=f"sc{b}")
                nc.tensor.matmul(sc[:], qT[b][:], kT[b][:], start=True,
                                 stop=True)
                sg = caan.tile([A, A], F32, tag="sg")
                nc.vector.scalar_tensor_tensor(sg[:], sc[:],
                                               1.0 / np.sqrt(ATTN),
                                               gates[b][:],
                                               op0=OP.mult, op1=OP.mult)
                asum = caan.tile([A, 1], F32, tag="asum")
                ae = caan.tile([A, A], F32, tag="ae")
                nc.scalar.activation(ae[:], sg[:], AF.Exp, accum_out=asum[:])
                arec = caan.tile([A, 1], F32, tag="arec")
                nc.vector.reciprocal(arec[:], asum[:])
                attn = caan.tile([A, A], BF16, tag="attn")
                nc.vector.tensor_scalar_mul(attn[:], ae[:], arec[:])
                atp = psc.tile([A, A], BF16, tag="c", name=f"atp{b}")
                nc.tensor.transpose(atp[:], attn[:], ident_b[:])
                attnT = caan.tile([A, A], BF16, tag="attnT")
                nc.vector.tensor_copy(attnT[:], atp[:])
                aop = psc.tile([ATTN, A], F32, tag="c", name=f"aop{b}")
                nc.tensor.matmul(aop[:], vb[b][:], attnT[:], start=True,
                                 stop=True)
                at = caan.tile([ATTN, A], BF16, tag=f"aoT{b}")
                nc.vector.tensor_copy(at[:], aop[:])
                aoT.append(at)
            # feed-forward + LN2 (rstd batched over b)
            f2s = []
            var4b = caan.tile([A, 4], F32, tag="var4b")
            for b in range(BPC):
                h1p = psc.tile([A, 2 * ATTN], F32, tag="c", name=f"h1p{b}")
                nc.tensor.matmul(h1p[:], aoT[b][:], ff1[:], start=True,
                                 stop=ff1b is None)
                if ff1b is not None:
                    nc.tensor.matmul(h1p[:], ones_1_128b[:], ff1b[:],
                                     start=False, stop=True)
                h1c = caan.tile([A, 2 * ATTN], BF16, tag="h1c")
                nc.scalar.activation(h1c[:], h1p[:], AF.Relu)
                h1tp = psc.tile([2 * ATTN, A], BF16, tag="c",
                                name=f"h1tp{b}")
                nc.tensor.transpose(h1tp[:], h1c[:], ident_b[:])
                h1T = caan.tile([2 * ATTN, A], BF16, tag="h1T")
                nc.vector.tensor_copy(h1T[:], h1tp[:])
                f2p = psc.tile([A, ATTN], F32, tag="c", name=f"f2p{b}")
                nc.tensor.matmul(f2p[:], h1T[:], ff2[:], start=True,
                                 stop=ff2b is None)
                if ff2b is not None:
                    nc.tensor.matmul(f2p[:], ones_1_128b[:], ff2b[:],
                                     start=False, stop=True)
                f2 = caan.tile([A, ATTN], F32, tag=f"f2{b}")
                nc.vector.tensor_copy(f2[:], f2p[:])
                st6b = caan.tile([A, nc.vector.BN_STATS_DIM], F32,
                                 tag="st6b")
                nc.vector.bn_stats(out=st6b[:], in_=f2[:])
                mvb = caan.tile([A, nc.vector.BN_AGGR_DIM], F32,
                                tag=f"mvb{b}")
                nc.vector.bn_aggr(out=mvb[:], in_=st6b[:])
                nc.vector.tensor_scalar_add(var4b[:, b:b + 1], mvb[:, 1:2],
                                            1e-5)
                f2s.append((f2, mvb))
            rstd4b = _rsqrt_quake(nc, caan, var4b[:], 4)
            for b in range(BPC):
                f2, mvb = f2s[b]
                lt = caan.tile([A, ATTN], F32, tag="ltb")
                nc.vector.tensor_scalar_sub(lt[:], f2[:], mvb[:, 0:1])
                lt2 = caan.tile([A, ATTN], F32, tag="ltb2")
                nc.vector.scalar_tensor_tensor(lt2[:], lt[:],
                                               rstd4b[:, b:b + 1],
                                               gbc2[:], op0=OP.mult,
                                               op1=OP.mult)
                ffo = caan.tile([A, ATTN], BF16, tag="ffo")
                nc.vector.tensor_add(ffo[:], lt2[:], bbc2[:])
                # scorer: sigmoid(sp2 @ relu(sp1 @ ff + b1) + b2) via exp
                fftp = psc.tile([ATTN, A], BF16, tag="c", name=f"fftp{b}")
                nc.tensor.transpose(fftp[:], ffo[:], ident_b[:])
                ffT = caan.tile([ATTN, A], BF16, tag="ffT")
                nc.vector.tensor_copy(ffT[:], fftp[:])
                s1p = psc.tile([32, A], F32, tag="c", name=f"s1p{b}")
                nc.tensor.matmul(s1p[:], sp1[:], ffT[:], start=True,
                                 stop=True)
                s1 = caan.tile([32, A], BF16, tag="s1")
                nc.scalar.activation(s1[:], s1p[:], AF.Relu, bias=sp1b[:])
                s2p = psc.tile([1, A], F32, tag="c", name=f"s2p{b}")
                nc.tensor.matmul(s2p[:], sp2[:], s1[:], start=True, stop=True)
                # sigmoid(z) = 1 / (1 + exp(-z)); nsp2b = -sp2_bias
                en = caan.tile([1, A], F32, tag="en")
                nc.scalar.activation(en[:], s2p[:], AF.Exp, bias=nsp2b[:],
                                     scale=-1.0)
                ep1 = caan.tile([1, A], F32, tag="ep1")
                nc.vector.tensor_scalar_add(ep1[:], en[:], 1.0)
                s2 = caan.tile([1, A], F32, tag="s2")
                nc.vector.reciprocal(s2[:], ep1[:])
                nc.sync.dma_start(out=out_d.ap()[b:b + 1, :], in_=s2[:])

    nc.compile()
    return nc


def _reord(w):
    """PyTorch gate order i,f,g,o kept as i,f,g,o (on last axis), with the
    g-gate block scaled x2 (tanh(g) = 2*sigmoid(2g) - 1)."""
    i, f, g, o = np.split(w, 4, axis=-1)
    return np.concatenate([i, f, 2.0 * g, o], axis=-1)


def kernel(**inp):
    x = np.asarray(inp["x"], np.float32)
    ranks = np.asarray(inp["ranks"], np.int32)

    def bf(a):
        return np.ascontiguousarray(np.asarray(a, np.float32).astype(BF))

    def f8(a):
        return np.clip(np.asarray(a, np.float32), -240, 240).astype(E4)

    w0t = _reord(np.asarray(inp["W_ih0"], np.float32).T)
    whh0 = _reord(np.asarray(inp["W_hh0"], np.float32).T)
    wih1 = _reord(np.asarray(inp["W_ih1"], np.float32).T)
    whh1 = _reord(np.asarray(inp["W_hh1"], np.float32).T)
    # fp8 DoubleRow packs [K, gate, ktile, M]: psA = wih0@x + whh0@h1,
    # psB = wih1@h1 + whh1@h2
    waf = np.zeros((A, 4, 2, H), np.float32)
    wbf = np.zeros((A, 4, 2, H), np.float32)
    for g in range(4):
        waf[0:D, g, 0, :] = w0t[:, g * H:(g + 1) * H]
        waf[:, g, 1, :] = whh0[:, g * H:(g + 1) * H]
        wbf[:, g, 0, :] = wih1[:, g * H:(g + 1) * H]
        wbf[:, g, 1, :] = whh1[:, g * H:(g + 1) * H]
    wa = np.ascontiguousarray(f8(waf).reshape(A, 4 * 2 * H))
    wb = np.ascontiguousarray(f8(wbf).reshape(A, 4 * 2 * H))
    b0v = np.asarray(inp["b_ih0"], np.float32) + np.asarray(inp["b_hh0"],
                                                            np.float32)
    b1v = np.asarray(inp["b_ih1"], np.float32) + np.asarray(inp["b_hh1"],
                                                            np.float32)
    b0 = bf(_reord(b0v)[None, :])
    b1 = bf(_reord(b1v)[None, :])

    # host-precomputed rank-distance gate table: gmat[p, q] = gate(|p-q|)
    emb = np.asarray(inp["rank_emb"], np.float32)
    rw1 = np.asarray(inp["rw1_W"], np.float32)
    rw1b = np.asarray(inp["rw1_b"], np.float32)
    rw2 = np.asarray(inp["rw2_W"], np.float32)
    gv = 1.0 / (1.0 + np.exp(-(np.maximum(emb @ rw1 + rw1b, 0.0) @ rw2)))
    pq = np.abs(np.arange(A)[:, None] - np.arange(A)[None, :])
    gmat = bf(gv[np.clip(pq, 0, MAX_DIST)])
    iotap = np.ascontiguousarray(
        np.broadcast_to(np.arange(A, dtype=np.int32)[:, None], (A, A)))

    has_b0 = bool(np.any(b0v))
    has_b1 = bool(np.any(b1v))
    has_bv = bool(np.any(np.asarray(inp["bv"], np.float32)))
    has_f1b = bool(np.any(np.asarray(inp["ff1_b"], np.float32)))
    has_f2b = bool(np.any(np.asarray(inp["ff2_b"], np.float32)))
    ck = (has_b0, has_b1, has_bv, has_f1b, has_f2b)
    if ck not in _cache:
        _cache[ck] = _build(*ck)
    nc = _cache[ck]

    shared = dict(
        wa=wa, wb=wb, b0=b0, b1=b1,
        aw1=bf(inp["attn_W1"]), aw2=bf(inp["attn_W2"]),
        awv=bf(np.asarray(inp["attn_w"], np.float32)[:, None]),
        ln1g=np.asarray(inp["ln1_g"], np.float32)[None, :].copy(),
        ln1b=np.asarray(inp["ln1_b"], np.float32)[None, :].copy(),
        projw=bf(inp["proj_W"]),
        projb=np.asarray(inp["proj_b"], np.float32)[:, None].copy(),
        wq=bf(inp["Wq"]), bq=np.asarray(inp["bq"], np.float32)[:, None].copy(),
        wk=bf(inp["Wk"]), bk=np.asarray(inp["bk"], np.float32)[:, None].copy(),
        wv=bf(inp["Wv"]), bv=bf(np.asarray(inp["bv"], np.float32)[None, :]),
        gmat=gmat, iotap=iotap,
        ff1=bf(inp["ff1_W"]),
        ff1b=bf(np.asarray(inp["ff1_b"], np.float32)[None, :]),
        ff2=bf(inp["ff2_W"]),
        ff2b=bf(np.asarray(inp["ff2_b"], np.float32)[None, :]),
        ln2g=np.asarray(inp["ln2_g"], np.float32)[None, :].copy(),
        ln2b=np.asarray(inp["ln2_b"], np.float32)[None, :].copy(),
        sp1=bf(inp["sp1_W"]),
        sp1b=np.asarray(inp["sp1_b"], np.float32)[:, None].copy(),
        sp2=bf(inp["sp2_W"]),
        nsp2b=(-np.asarray(inp["sp2_b"], np.float32))[None, :].copy(),
    )

    in_maps = []
    for c in range(N_CORES):
        xc = x[c * BPC:(c + 1) * BPC].reshape(S, T, D).transpose(1, 2, 0)
        m = dict(shared)
        m["x"] = np.ascontiguousarray(f8(xc))
        m["ranks"] = np.ascontiguousarray(ranks[c * BPC:(c + 1) * BPC])
        in_maps.append(m)

    global _last_in_maps
    _last_in_maps = in_maps
    res = run_bass_kernel_spmd(nc, in_maps, core_ids=list(range(N_CORES)))
    out = np.concatenate([res.results[c]["out"] for c in range(N_CORES)],
                         axis=0)
    return out.astype(np.float32)
